# revision 50
# baseline (speedup 1.0000x reference)
"""Grouped Conv2d (512 groups, 2->2 ch/group, 3x3 VALID) on 8 trn2 NeuronCores.

Band-matrix formulation. Both x and the banded weights travel as
fp8e3m4 (weights pre-scaled by 32 to clear the subnormal range; the
1/32 is folded into the PSUM eviction), which keeps the DMA engines
(~58us of traffic) under the PE floor (~69us). Batches run as 4 fused
quads with four batches in the matmul moving dim (216 columns), so the
PE consumes weight chunks at half the rate they arrive and the startup
weight stream never stalls it. Weight chunks and the first quad's x
slices live in single-writer tiles (the dependency tracker makes a
reader wait on one write past its own region otherwise). PE p-state
ramps on dummy matmuls that only depend on an SBUF memset, burning the
initial DMA latency. Steady-state y stores issue from the otherwise-
idle Pool (SWDGE) queue so the eviction engines' sequencers never
block on DMA semaphore waits; the last blocks ship as small pieces
from the HWDGE queues (SWDGE's 1us descriptor-gen would serialize the
tail).
"""

import sys

import numpy as np

for _p in ("/opt/trn_rl_repo",):
    if _p not in sys.path:
        sys.path.insert(0, _p)

import ml_dtypes

import concourse.bacc as bacc
import concourse.bass as bass
import concourse.tile as tile
from concourse import mybir
from concourse.bass_utils import run_bass_kernel_spmd

N_CORES = 8
B, C, H, W = 16, 1024, 56, 56
NQ = B // 4  # 4 fused quads
NB = 4  # batches per quad
KH = KW = 3
HO, WO = H - KH + 1, W - KW + 1  # 54, 54
CPC = C // N_CORES  # 128 channels per core
G = CPC // 2  # 64 groups per core
P_IN = 2 * H  # 112 partitions: (ic, dy)
P_OUT = 2 * HO  # 108 lanes: (oc, oy)
GPT = 2  # groups per psum tile (2*4*54 = 432 fp32, one bank)
NGB = G // GPT  # 32 blocks per quad
WCH = 8  # groups per weight tile/DMA chunk
WSCALE = 32.0  # fp8 weight pre-scale (undone at eviction)
N_DUMMY = 10  # big PE p-state ramp matmuls during startup DMA latency
N_DUMS = 0  # small trailing ramp matmuls
WB0 = 8  # first weight chunk groups

BF16 = ml_dtypes.bfloat16
F8E3 = ml_dtypes.float8_e3m4

_NC_CACHE = {}


def _build_program():
    nc = bacc.Bacc(
        "TRN2", target_bir_lowering=False, debug=False, num_devices=N_CORES
    )
    f32 = mybir.dt.float32
    bf16 = mybir.dt.bfloat16
    f8 = mybir.dt.float8e3

    xp_d = nc.declare_dram_parameter(
        "xp", [2, P_IN, G, NB, W], f8, isOutput=False
    )
    # Quads 2-3 ride bf16: the extra DMA hides under the PE floor and
    # widens the fp8 error margin. Quads 0-1 stay fp8 so the startup
    # stream and the first inter-quad DMA window keep ahead of the PE.
    xb_d = nc.declare_dram_parameter(
        "xb", [NQ - 2, P_IN, G, NB, W], bf16, isOutput=False
    )
    wb_d = nc.declare_dram_parameter(
        "wb", [P_IN, G, KW, P_OUT], f8, isOutput=False
    )
    yp_d = nc.declare_dram_parameter(
        "yp", [NQ, P_OUT, G, NB, WO], bf16, isOutput=True
    )

    with tile.TileContext(nc) as tc:
        with (
            tc.tile_pool(name="wpool", bufs=1) as wpool,
            tc.tile_pool(name="x0pool", bufs=1) as x0pool,
            tc.tile_pool(name="xqpool", bufs=2) as xqpool,
            tc.tile_pool(name="oqpool", bufs=2) as oqpool,
            tc.tile_pool(name="psum", bufs=8, space="PSUM") as ppool,
        ):
            # Single-writer weight tiles: one per DMA chunk. A 6-group
            # first chunk unlocks the first matmul earlier; the rest are
            # 8-group (smaller chunks leave HWDGE-issue bubbles on the
            # DMA engines: one DMA can only launch every ~630ns).
            wsl = [(0, WB0)] + [
                (lo, min(lo + WCH, G)) for lo in range(WB0, G, WCH)
            ]
            wts = [
                wpool.tile([P_IN, hi - lo, KW, P_OUT], f8, name=f"wt{i}")
                for i, (lo, hi) in enumerate(wsl)
            ]
            dum = wpool.tile([P_IN, KW * P_OUT], bf16)
            _emit(nc, tc, x0pool, xqpool, oqpool, ppool,
                  xp_d, xb_d, yp_d, wb_d, wsl, wts, dum)
    nc.compile()
    return nc


def _emit(nc, tc, x0pool, xqpool, oqpool, ppool, xp_d, xb_d, yp_d, wb_d,
          wsl, wts, dum):
    f32 = mybir.dt.float32
    bf16 = mybir.dt.bfloat16
    f8 = mybir.dt.float8e3

    # Dummy-ramp source: memset so the dummies depend on nothing external.
    nc.vector.memset(dum[:], 0)

    # Quad 0 arrives as single-writer tiles in doubling slices so early
    # blocks wait only on their own slice; quads 1-3 are one tile each.
    X0SL = [(0, 4), (4, 12), (12, 28), (28, 64)]
    x0ts = [
        x0pool.tile([P_IN, hi - lo, NB, W], f8, name=f"x0t{i}")
        for i, (lo, hi) in enumerate(X0SL)
    ]

    def x0view(g):
        for (lo, hi), t in zip(X0SL, x0ts):
            if g < hi:
                return t, g - lo
        raise AssertionError

    xqts = {}

    def load_xq(q):
        if q == 1:
            xqts[q] = xqpool.tile([P_IN, G, NB, W], f8, name="xq")
            nc.sync.dma_start(out=xqts[q][:], in_=xp_d[1])
        else:
            xqts[q] = xqpool.tile([P_IN, G, NB, W], bf16, name="xq")
            nc.sync.dma_start(out=xqts[q][:], in_=xb_d[q - 2])

    # Startup DMA order (all SP queue; transfers serialize on the DMA
    # engines in this order): the first weight slice and quad 0's
    # first 4 groups gate the first real matmul; the rest of the
    # weights and quad 0 interleave with margins at quad pacing.
    def wview(g):
        for i, (lo, hi) in enumerate(wsl):
            if g < hi:
                return wts[i], g - lo
        raise AssertionError

    def load_wb(i):
        lo, hi = wsl[i]
        nc.sync.dma_start(out=wts[i][:], in_=wb_d[:, lo:hi])

    def load_x0(i):
        lo, hi = X0SL[i]
        nc.sync.dma_start(out=x0ts[i][:], in_=xp_d[0, :, lo:hi])

    load_wb(0)
    load_x0(0)
    load_x0(1)
    load_wb(1)
    load_x0(2)
    load_wb(2)
    load_wb(3)
    load_x0(3)
    for i in range(4, len(wsl)):
        load_wb(i)

    # PE p-state ramp through the startup DMA latency: ends just past
    # the first real matmul's data arrival so the busy run is unbroken
    # and the clock is at full speed for all real work. The short final
    # dummies quantize the landing to ~0.1us.
    for ncols in [KW * P_OUT] * N_DUMMY + [P_OUT] * N_DUMS:
        scr = ppool.tile([P_OUT, 432], f32, name="pt")
        nc.tensor.matmul(
            scr[:, :ncols], lhsT=dum[:, :P_OUT], rhs=dum[:, :ncols],
            start=True, stop=True,
        )

    oqts = {}

    def emit_quad(q, gb, g0=None, ng=GPT):
        """One block (ng groups) of a fused batch quad (216-col matmuls)."""
        if gb == 0:
            oqts[q] = oqpool.tile([P_OUT, G, NB, WO], bf16, name="oq")
            if q + 1 < NQ:
                load_xq(q + 1)
        if g0 is None:
            g0 = gb * GPT
        ot = oqts[q]
        pt = ppool.tile([P_OUT, 216 * ng], f32, name="pt")
        for gl in range(ng):
            g = g0 + gl
            if q == 0:
                xc, gx = x0view(g)
            else:
                xc, gx = xqts[q], g
            wc, gw = wview(g)
            for kw in range(KW):
                nc.tensor.matmul(
                    pt[:, gl * NB * WO:(gl + 1) * NB * WO],
                    lhsT=wc[:, gw, kw, :],
                    rhs=xc[:, gx, :, kw:kw + WO],
                    start=(kw == 0),
                    stop=(kw == KW - 1),
                )
        dst = ot[:, g0:g0 + ng, :, :]
        last = q == NQ - 1 and g0 + ng == G
        if last:
            # Final 1-group eviction on DVE (the prior one went to Act
            # in parallel), then the last 4 groups in one SP (HWDGE)
            # piece; SWDGE's serial descriptor-gen is too slow for the
            # drain, and splitting this piece loses: the first half's
            # read is write-tracked against the final eviction anyway
            # and the halves serialize on SEQ/HWDGE.
            nc.vector.tensor_scalar_mul(dst, pt[:], 1.0 / WSCALE)
            nc.sync.dma_start(
                out=yp_d[q, :, G - 2 * GPT:, :, :],
                in_=ot[:, G - 2 * GPT:, :, :],
            )
            xqts.pop(q, None)
            oqts.pop(q)
            return
        elif q == NQ - 1 and g0 == G - 2:
            nc.scalar.activation(
                dst, pt[:], mybir.ActivationFunctionType.Copy,
                scale=1.0 / WSCALE,
            )
            return
        elif gb % 2 == 0:
            nc.vector.tensor_scalar_mul(dst, pt[:], 1.0 / WSCALE)
        else:
            nc.scalar.activation(
                dst, pt[:], mybir.ActivationFunctionType.Copy,
                scale=1.0 / WSCALE,
            )
        # Steady-state y ships per 16-group quarter from the Pool
        # (SWDGE) queue: its sequencer has nothing else to do, so the
        # eviction engines never stall behind a DMA's semaphore wait.
        # The last quad's fourth quarter goes as small pieces from the
        # HWDGE queues instead (SWDGE descriptor-gen is ~1us serial on
        # the Pool engine, too slow for the drain).
        Q = G // 4
        if q == NQ - 1 and gb >= NGB - 8:
            b0 = gb * GPT
            if gb in (NGB - 7, NGB - 5, NGB - 3):
                # 4-group pieces while matmuls still run (Pool is fine
                # mid-flight; its 1us descriptor-gen overlaps compute).
                nc.gpsimd.dma_start(
                    out=yp_d[q, :, b0 - GPT:b0 + GPT, :, :],
                    in_=ot[:, b0 - GPT:b0 + GPT, :, :],
                )
        elif gb % 8 == 7:
            qq = gb // 8
            nc.gpsimd.dma_start(
                out=yp_d[q, :, qq * Q:(qq + 1) * Q, :, :],
                in_=ot[:, qq * Q:(qq + 1) * Q, :, :],
            )
            if gb == NGB - 1:
                xqts.pop(q, None)
                oqts.pop(q)

    for q in range(NQ):
        for gb in range(NGB - 1):
            emit_quad(q, gb)
        if q < NQ - 1:
            emit_quad(q, NGB - 1)
        else:
            # Last quad: split the final block into two 1-group blocks
            # so the drain-critical eviction is half as long and the
            # two evictions run on different engines concurrently.
            emit_quad(q, NGB - 1, g0=G - 2, ng=1)
            emit_quad(q, NGB - 1, g0=G - 1, ng=1)


def _get_nc():
    if "nc" not in _NC_CACHE:
        _NC_CACHE["nc"] = _build_program()
    return _NC_CACHE["nc"]


def _make_bands(w):
    """Per-core banded lhsT weights, shape (112, 64, 3, 108) fp8e3m4.

    bands[ic*56 + oy + kh, g, kw, oc*54 + oy] = WSCALE * w[2g+oc, ic, kh, kw]
    """
    w = np.asarray(w, dtype=np.float32)
    wg = w.reshape(G * N_CORES, 2, 2, KH, KW)  # [g_all, oc, ic, kh, kw]
    oy = np.arange(HO)
    mats = []
    for cid in range(N_CORES):
        ws = np.clip(wg[cid * G:(cid + 1) * G] * WSCALE, -15.5, 15.5)
        bands = np.zeros((P_IN, G, KW, P_OUT), dtype=np.float32)
        for ic in range(2):
            for oc in range(2):
                for kh in range(KH):
                    bands[ic * H + oy + kh, :, :, oc * HO + oy] = (
                        ws[:, oc, ic, kh, :][None, :, :]
                    )
        mats.append(bands.astype(F8E3))
    return mats


def _permute_x(x):
    """Full x -> per-core quad layout x[q,(ic,dy),g,nb,j].

    Quad 0 is fp8e3m4 (startup stream), quads 1-3 bf16."""
    x = np.asarray(x)
    out = []
    for cid in range(N_CORES):
        xs = x[:, cid * CPC:(cid + 1) * CPC]
        xg = xs.reshape(NQ, NB, G, 2, H, W).transpose(0, 3, 4, 2, 1, 5)
        xg = np.ascontiguousarray(xg.reshape(NQ, P_IN, G, NB, W))
        out.append((xg[:2].astype(F8E3), xg[2:].astype(BF16)))
    return out


def _unpermute_y(res):
    """Per-core quad outputs -> full f32 NCHW."""
    parts = []
    for cid in range(N_CORES):
        yq = np.asarray(res[cid]["yp"]).astype(np.float32)
        yqc = yq.reshape(NQ, 2, HO, G, NB, WO).transpose(0, 4, 3, 1, 2, 5)
        parts.append(yqc.reshape(B, CPC, HO, WO))
    return np.concatenate(parts, axis=1)


def _run(x, w, trace=False, **kwargs):
    nc = _get_nc()
    xperm = _permute_x(x)
    bands = _make_bands(w)
    in_maps = [
        {"xp": xperm[cid][0], "xb": xperm[cid][1], "wb": bands[cid]}
        for cid in range(N_CORES)
    ]
    res = run_bass_kernel_spmd(
        nc, in_maps, list(range(N_CORES)), trace=trace, **kwargs
    )
    y = _unpermute_y(res.results)
    return y, res


def kernel(x, w):
    y, _ = _run(x, w, trace=False)
    return y
